# revision 28
# baseline (speedup 1.0000x reference)
"""HAN layer (3-metapath GraphConv + semantic attention) on 8 trn2 NeuronCores.

Strategy (per sharding hint): shard destination nodes across the 8 cores
(6250 rows each), partition each metapath's edge list by destination shard
on the host, and sort/pad it into fixed-size 128-edge chunks per 128-dst
output tile.  Each core gathers source rows of h (stored bf16, replicated
in every core's DRAM) with the vectorized SWDGE ``dma_gather`` instruction
(<=1024 indices per call -- larger calls overflow the SWDGE descriptor
ring; src ids are split at 32768 into two base pointers because the
instruction's indices are int16), multiplies each gathered 128-edge chunk
with a host-precomputed selection matrix S (streamed from DRAM in bf16;
S[edge, dst_rel] = norm weight) on the tensor engine, accumulating z^T
tiles in PSUM.  A tiny [1,3] per-metapath score vector is all-reduced
across cores for the softmax semantic attention, then each core writes its
6250-row slice of the output.
"""

import numpy as np
import ml_dtypes

import concourse.bass as bass
import concourse.bacc as bacc
import concourse.mybir as mybir
import concourse.tile as tile
from concourse.bass_utils import run_bass_kernel_spmd

P = 128
N = 50000
D = 128
M = 3
E = 1_600_000
NCORES = 8
NSH = N // NCORES          # 6250 dst rows per core
NTILES = (NSH + P - 1) // P  # 49 output tiles (last has 106 real rows)
SPLIT = 32768              # int16 gather-index limit
GMAX = 8                   # max chunks (1024 idxs) per dma_gather call
DFRAC = 0.6                # fraction of S chunks streamed from DRAM
                           # (the rest are built on the idle vector engine)

TRACE = False
LAST_RESULTS = None

_PROGRAM_CACHE = {}


def _wrap_idx(idx):
    """[n] int16 -> [128, n//16] SWDGE index layout: position j at
    [j % 16, j // 16], replicated across the 8 groups of 16 partitions."""
    n = idx.shape[0]
    arr = idx.reshape(n // 16, 16).T  # [16, n//16]
    return np.tile(arr, (8, 1))


def _preprocess(edges):
    """Host-side: per-core gather-index streams and selection matrices.

    Per core, per dst tile t (128 dst rows), edges of all 3 metapaths are
    split into src<SPLIT ("lo") and src>=SPLIT ("hi") halves, each half
    grouped per metapath, sorted by src, and padded to a COMMON
    (max-over-cores) multiple-of-128 segment size so the SPMD program's
    layout is identical on every core (pad idx 0, wgt 0).
    Chunk order within a tile: [lo m0, lo m1, lo m2, hi m0, hi m1, hi m2].

    Returns (idx_all, s_all, cfg):
      idx_all[core]: [128, NTILES*W_IDX] int16, per-t window holds the
        wrapped lo indices then hi indices (hi stored as src-SPLIT).
      s_all[core]: [128, NTILES*NCH*128] bf16 selection matrices;
        chunk c of tile t at cols (t*NCH+c)*128: S[p, drel] = wgt for
        edge (c*128+p).
      cfg: (NCH, W_IDX, seg chunk counts per (t, half, m)).
    """
    per_core = [[None] * M for _ in range(NCORES)]
    for m in range(M):
        src = np.asarray(edges[m, 0])
        dst = np.asarray(edges[m, 1])
        out_deg = np.bincount(src, minlength=N).astype(np.float32)
        in_deg = np.bincount(dst, minlength=N).astype(np.float32)
        ns = 1.0 / np.sqrt(np.maximum(out_deg, 1.0))
        nd = 1.0 / np.sqrt(np.maximum(in_deg, 1.0))
        w_e = (ns[src] * nd[dst]).astype(np.float32)

        order = np.lexsort((src, dst))
        src_s, dst_s, w_s = src[order], dst[order], w_e[order]
        shard_bounds = np.searchsorted(dst_s, NSH * np.arange(NCORES + 1))
        for core in range(NCORES):
            lo, hi = shard_bounds[core], shard_bounds[core + 1]
            per_core[core][m] = (
                src_s[lo:hi],
                dst_s[lo:hi] - core * NSH,
                w_s[lo:hi],
            )

    # Raw per (core, t, half, m) segments: unique sorted src ids to gather,
    # plus per-edge (position-in-unique, drel, wgt) for the S scatter.
    segs = {}
    seg_n = np.zeros((NCORES, NTILES, 2, M), np.int64)
    for core in range(NCORES):
        for m in range(M):
            sc, dc, wc = per_core[core][m]
            tile_id = dc >> 7
            tb = np.searchsorted(tile_id, np.arange(NTILES + 1))
            for t in range(NTILES):
                s = slice(tb[t], tb[t + 1])
                st, dt_, wt = sc[s], dc[s] - t * P, wc[s]
                lo_mask = st < SPLIT
                for half in range(2):
                    mk = lo_mask if half == 0 else ~lo_mask
                    ss, ds, ws = st[mk], dt_[mk], wt[mk]
                    if half == 0:
                        # DRAM-S path: dedupe by src (multi-dst lanes OK)
                        uniq, inv = np.unique(ss, return_inverse=True)
                        segs[(core, t, half, m)] = (uniq, inv, ds, ws)
                        seg_n[core, t, half, m] = uniq.shape[0]
                    else:
                        # DVE-built-S path: one edge per lane (src-sorted)
                        o = np.argsort(ss, kind="stable")
                        segs[(core, t, half, m)] = (
                            ss[o],
                            None,
                            ds[o],
                            ws[o],
                        )
                        seg_n[core, t, half, m] = ss.shape[0]

    # Common (max over cores) chunk count per (t, half, m).
    segc = -(-seg_n.max(axis=0) // P)  # [NTILES, 2, M] ceil
    NCH = int(segc.sum(axis=(1, 2)).max())
    nidx_t = segc.sum(axis=(1, 2)) * P
    W_IDX = int(nidx_t.max()) // 16
    ND = int(segc[:, 0, :].sum(axis=1).max())  # DRAM-S (lo) chunks per tile
    NW = int(segc[:, 1, :].sum(axis=1).max())  # DVE-S (hi) chunks per tile

    idx_all, s_all, wd_all = [], [], []
    for core in range(NCORES):
        idx_arr = np.zeros((128, NTILES * W_IDX), np.int16)
        s_arr = np.zeros((128, NTILES * ND * 128), ml_dtypes.bfloat16)
        wd_arr = np.zeros((128, NTILES * 2 * NW), np.float32)
        for t in range(NTILES):
            parts = []
            c_lo = 0
            c_hi = 0
            for half in range(2):
                for m in range(M):
                    uniq, inv, ds, ws = segs[(core, t, half, m)]
                    ncol = int(segc[t, half, m])
                    npad = ncol * P - uniq.shape[0]
                    base = 0 if half == 0 else SPLIT
                    up = np.concatenate(
                        [uniq, np.full(npad, base, uniq.dtype)]
                    ).astype(np.int64)
                    parts.append((up - base).astype(np.int16))
                    if half == 0:
                        # S scatter: unique-pos q -> chunk q//128, lane
                        # q%128; duplicate (src, dst) edges accumulate.
                        seg = np.zeros((128, ncol * 128), np.float32)
                        np.add.at(seg, (inv % P, (inv // P) * P + ds), ws)
                        s_arr[
                            :,
                            (t * ND + c_lo)
                            * 128 : (t * ND + c_lo + ncol)
                            * 128,
                        ] = seg.astype(ml_dtypes.bfloat16)
                        c_lo += ncol
                    else:
                        # compact (wgt, drel); one edge per lane
                        dsp = np.concatenate([ds, np.zeros(npad, ds.dtype)])
                        wsp = np.concatenate([ws, np.zeros(npad, ws.dtype)])
                        wbase = t * 2 * NW
                        wd_arr[:, wbase + c_hi : wbase + c_hi + ncol] = (
                            wsp.reshape(ncol, P).T
                        )
                        wd_arr[
                            :,
                            wbase + NW + c_hi : wbase + NW + c_hi + ncol,
                        ] = dsp.reshape(ncol, P).T.astype(np.float32)
                        c_hi += ncol
            flat = np.concatenate(parts)
            idx_arr[:, t * W_IDX : t * W_IDX + flat.shape[0] // 16] = _wrap_idx(
                flat
            )
        idx_all.append(np.ascontiguousarray(idx_arr))
        s_all.append(np.ascontiguousarray(s_arr))
        wd_all.append(np.ascontiguousarray(wd_arr))

    cfg = (
        NCH,
        W_IDX,
        tuple(
            tuple(tuple(int(x) for x in segc[t, h]) for h in range(2))
            for t in range(NTILES)
        ),
    )
    return idx_all, s_all, wd_all, cfg


def _build_program(cfg):
    if cfg in _PROGRAM_CACHE:
        return _PROGRAM_CACHE[cfg]
    f32 = mybir.dt.float32
    bf16 = mybir.dt.bfloat16
    NCH, W_IDX, segc_t = cfg
    nlo_t = [sum(segc_t[t][0]) * P for t in range(NTILES)]
    nhi_t = [sum(segc_t[t][1]) * P for t in range(NTILES)]
    ND = max(sum(segc_t[t][0]) for t in range(NTILES))
    NW = max(sum(segc_t[t][1]) for t in range(NTILES))

    nc = bacc.Bacc(
        None, target_bir_lowering=False, num_devices=NCORES, num_swdge_queues=4
    )
    h_d = nc.dram_tensor("h", [N, D], bf16, kind="ExternalInput")
    idx_d = nc.dram_tensor(
        "idx", [128, NTILES * W_IDX], mybir.dt.int16, kind="ExternalInput"
    )
    s_d = nc.dram_tensor(
        "s", [128, NTILES * ND * 128], bf16, kind="ExternalInput"
    )
    wd_d = nc.dram_tensor(
        "wd", [128, NTILES * 2 * NW], f32, kind="ExternalInput"
    )
    iota_d = nc.dram_tensor("iota", [P, P], bf16, kind="ExternalInput")
    ident_d = nc.dram_tensor("ident", [P, P], bf16, kind="ExternalInput")
    w1_d = nc.dram_tensor("w1", [D, D], bf16, kind="ExternalInput")
    w2_d = nc.dram_tensor("w2", [D, 1], bf16, kind="ExternalInput")
    b1_d = nc.dram_tensor("b1", [D, 1], f32, kind="ExternalInput")
    out_d = nc.dram_tensor("out", [NSH, D], f32, kind="ExternalOutput")

    with tile.TileContext(nc) as tc:
        with (
            tc.tile_pool(name="consts", bufs=1) as cpool,
            tc.tile_pool(name="zpool", bufs=1) as zpool,
            tc.tile_pool(name="score_psum", bufs=1, space="PSUM") as sp,
        ):
          with (
            tc.tile_pool(name="meta", bufs=3) as mpool,
            tc.tile_pool(name="gather", bufs=2) as gpool,
            tc.tile_pool(name="sel", bufs=2) as spool,
            tc.tile_pool(name="psum_z", bufs=2, space="PSUM") as pz,
            tc.tile_pool(name="att_psum", bufs=2, space="PSUM") as ap_,
            tc.tile_pool(name="att_sbuf", bufs=2) as asb,
            tc.tile_pool(name="sdve", bufs=4) as dvp,
          ):
            ident_sb = cpool.tile([P, P], bf16)
            nc.sync.dma_start(out=ident_sb[:], in_=ident_d[:])
            iota_sb = cpool.tile([P, P], bf16)
            nc.sync.dma_start(out=iota_sb[:], in_=iota_d[:])
            w1_sb = cpool.tile([P, P], bf16)
            nc.sync.dma_start(out=w1_sb[:], in_=w1_d[:])
            w2_sb = cpool.tile([P, 1], bf16)
            nc.sync.dma_start(out=w2_sb[:], in_=w2_d[:])
            b1_sb = cpool.tile([P, 1], f32)
            nc.sync.dma_start(out=b1_sb[:], in_=b1_d[:])

            # z stored d-major: z^T[m] tile t lives at columns (m*NTILES+t)*P
            z_sb = zpool.tile([P, M * NTILES * P], bf16)

            psum_s = sp.tile([1, M * P], f32, space="PSUM")
            qrr = [0]  # SWDGE queue round-robin counter
            # ---- sparse aggregation ----
            for t in range(NTILES):
                nlo, nhi = nlo_t[t], nhi_t[t]
                nch = (nlo + nhi) // P
                nlo_ch = nlo // P
                i_sb = mpool.tile([128, W_IDX], mybir.dt.int16, tag="idx")
                nc.sync.dma_start(
                    out=i_sb[:], in_=idx_d[:, t * W_IDX : (t + 1) * W_IDX]
                )
                s_sb = spool.tile([128, ND * 128], bf16, tag="s")
                nc.sync.dma_start(
                    out=s_sb[:, : nlo_ch * 128],
                    in_=s_d[:, t * ND * 128 : t * ND * 128 + nlo_ch * 128],
                )
                wd_sb = mpool.tile([128, 2 * NW], f32, tag="wd")
                nc.sync.dma_start(
                    out=wd_sb[:], in_=wd_d[:, t * 2 * NW : (t + 1) * 2 * NW]
                )
                g_sb = gpool.tile([128, nch, P], bf16, tag="g")
                # dma_gather is limited to 1024 indices (8 chunks) per call:
                # larger calls overflow the SWDGE descriptor ring and wedge
                # the device.  Round-robin the 4 SWDGE queues.
                for half, base_ap, ch0, nch_h in (
                    (0, h_d[:, :], 0, nlo // P),
                    (1, h_d[SPLIT:, :], nlo // P, nhi // P),
                ):
                    for off in range(0, nch_h, GMAX):
                        k = min(GMAX, nch_h - off)
                        c = ch0 + off
                        nc.gpsimd.dma_gather(
                            g_sb[:, c : c + k, :],
                            base_ap,
                            i_sb[:, c * 8 : c * 8 + k * 8],
                            k * P,
                            k * P,
                            P,
                            queue_num=qrr[0] % 4,
                        )
                        qrr[0] += 1
                # chunk columns for (half, m) segments in order
                seg_off = {}
                c0 = 0
                for half in range(2):
                    for m in range(M):
                        seg_off[(half, m)] = c0
                        c0 += segc_t[t][half][m]
                for m in range(M):
                    cols = list(
                        range(
                            seg_off[(0, m)],
                            seg_off[(0, m)] + segc_t[t][0][m],
                        )
                    ) + list(
                        range(
                            seg_off[(1, m)],
                            seg_off[(1, m)] + segc_t[t][1][m],
                        )
                    )
                    psum_zt = pz.tile([P, P], f32, space="PSUM", tag="zt")
                    for k, c in enumerate(cols):
                        if c < nlo_ch:
                            rhs = s_sb[:, c * 128 : (c + 1) * 128]
                        else:
                            ch = c - nlo_ch
                            s_dve = dvp.tile([P, P], bf16, tag="sd")
                            nc.vector.tensor_scalar(
                                out=s_dve[:],
                                in0=iota_sb[:],
                                scalar1=wd_sb[:, NW + ch : NW + ch + 1],
                                scalar2=wd_sb[:, ch : ch + 1],
                                op0=mybir.AluOpType.is_equal,
                                op1=mybir.AluOpType.mult,
                            )
                            rhs = s_dve[:]
                        nc.tensor.matmul(
                            out=psum_zt[:],
                            lhsT=g_sb[:, c, :],
                            rhs=rhs,
                            start=(k == 0),
                            stop=(k == len(cols) - 1),
                        )
                    zt = z_sb[
                        :, (m * NTILES + t) * P : (m * NTILES + t + 1) * P
                    ]
                    nc.scalar.copy(out=zt, in_=psum_zt[:])
                    # fold the semantic-attention score accumulation in so
                    # it overlaps the gather-bound main loop
                    psum_y = ap_.tile([P, P], f32, space="PSUM", tag="y")
                    nc.tensor.matmul(
                        out=psum_y[:], lhsT=w1_sb[:], rhs=zt, start=True,
                        stop=True,
                    )
                    tanh_sb = asb.tile([P, P], bf16, tag="tanh")
                    nc.scalar.activation(
                        out=tanh_sb[:],
                        in_=psum_y[:],
                        func=mybir.ActivationFunctionType.Tanh,
                        bias=b1_sb[:, :1],
                    )
                    nc.tensor.matmul(
                        out=psum_s[:, m * P : (m + 1) * P],
                        lhsT=w2_sb[:],
                        rhs=tanh_sb[:],
                        start=(t == 0),
                        stop=(t == NTILES - 1),
                        skip_group_check=True,
                    )

          # ---- semantic attention epilogue ----
          with (
            tc.tile_pool(name="out_psum", bufs=2, space="PSUM") as op_,
            tc.tile_pool(name="epi_sbuf", bufs=2) as es,
            tc.tile_pool(name="small", bufs=1) as sm,
          ):
            wrow = sm.tile([1, M], f32)
            for m in range(M):
                nc.vector.reduce_sum(
                    out=wrow[:, m : m + 1],
                    in_=psum_s[:, m * P : (m + 1) * P],
                    axis=mybir.AxisListType.X,
                )
            with tc.tile_pool(name="ccdram", bufs=1, space="DRAM") as ccp:
                cc_in_t = ccp.tile([1, M], f32)
                cc_out_t = ccp.tile([1, M], f32, addr_space="Shared")
                nc.gpsimd.dma_start(cc_in_t[:], wrow[:])
                nc.gpsimd.collective_compute(
                    "AllReduce",
                    mybir.AluOpType.add,
                    replica_groups=[list(range(NCORES))],
                    ins=[cc_in_t.opt()],
                    outs=[cc_out_t.opt()],
                )
                w_bc = sm.tile([P, M], f32)
                nc.sync.dma_start(
                    out=w_bc[:], in_=cc_out_t[0:1, :].to_broadcast([P, M])
                )
            # softmax over the M columns (identical on every partition)
            nc.vector.tensor_scalar(
                out=w_bc[:],
                in0=w_bc[:],
                scalar1=1.0 / N,
                scalar2=None,
                op0=mybir.AluOpType.mult,
            )
            negmax = sm.tile([P, 1], f32)
            nc.vector.tensor_reduce(
                out=negmax[:],
                in_=w_bc[:],
                axis=mybir.AxisListType.X,
                op=mybir.AluOpType.max,
                negate=True,
            )
            e_bc = sm.tile([P, M], f32)
            nc.scalar.activation(
                out=e_bc[:],
                in_=w_bc[:],
                func=mybir.ActivationFunctionType.Exp,
                bias=negmax[:, :1],
            )
            esum = sm.tile([P, 1], f32)
            nc.vector.reduce_sum(
                out=esum[:], in_=e_bc[:], axis=mybir.AxisListType.X
            )
            rsum = sm.tile([P, 1], f32)
            nc.vector.reciprocal(out=rsum[:], in_=esum[:])
            beta = sm.tile([P, M], f32)
            nc.vector.tensor_scalar(
                out=beta[:],
                in0=e_bc[:],
                scalar1=rsum[:, :1],
                scalar2=None,
                op0=mybir.AluOpType.mult,
            )
            ibeta = sm.tile([P, M * P], bf16)
            for m in range(M):
                nc.vector.tensor_scalar(
                    out=ibeta[:, m * P : (m + 1) * P],
                    in0=ident_sb[:],
                    scalar1=beta[:, m : m + 1],
                    scalar2=None,
                    op0=mybir.AluOpType.mult,
                )
            # ---- final combine: out tile = sum_m z_m^T(tile)^T @ (I * beta_m) ----
            for t in range(NTILES):
                psum_o = op_.tile([P, P], f32, space="PSUM", tag="o")
                for m in range(M):
                    nc.tensor.matmul(
                        out=psum_o[:],
                        lhsT=z_sb[:, (m * NTILES + t) * P : (m * NTILES + t + 1) * P],
                        rhs=ibeta[:, m * P : (m + 1) * P],
                        start=(m == 0),
                        stop=(m == M - 1),
                    )
                rows = min(P, NSH - t * P)
                o_sb = es.tile([P, P], f32, tag="out")
                nc.vector.tensor_copy(out=o_sb[:], in_=psum_o[:])
                nc.sync.dma_start(
                    out=out_d[t * P : t * P + rows, :], in_=o_sb[:rows, :]
                )
    nc.finalize()
    _PROGRAM_CACHE[cfg] = nc
    return nc


def kernel(h, edges, W1, b1, W2):
    global LAST_RESULTS
    h = np.asarray(h, dtype=np.float32)
    h_bf = np.ascontiguousarray(h.astype(ml_dtypes.bfloat16))
    edges = np.asarray(edges)
    idx_all, s_all, wd_all, cfg = _preprocess(edges)
    nc = _build_program(cfg)

    iota = np.tile(np.arange(P, dtype=np.float32), (P, 1)).astype(
        ml_dtypes.bfloat16
    )
    ident = np.eye(P, dtype=np.float32).astype(ml_dtypes.bfloat16)
    w1 = np.ascontiguousarray(
        np.asarray(W1, dtype=np.float32).astype(ml_dtypes.bfloat16)
    )
    w2 = np.ascontiguousarray(
        np.asarray(W2, dtype=np.float32).reshape(D, 1).astype(ml_dtypes.bfloat16)
    )
    b1c = np.ascontiguousarray(np.asarray(b1, dtype=np.float32).reshape(D, 1))

    in_maps = []
    for core in range(NCORES):
        in_maps.append(
            {
                "h": h_bf,
                "idx": idx_all[core],
                "s": s_all[core],
                "wd": wd_all[core],
                "iota": iota,
                "ident": ident,
                "w1": w1,
                "w2": w2,
                "b1": b1c,
            }
        )
    res = run_bass_kernel_spmd(
        nc, in_maps, core_ids=list(range(NCORES)), trace=TRACE
    )
    LAST_RESULTS = res
    out = np.concatenate([res.results[c]["out"] for c in range(NCORES)], axis=0)
    return out


# revision 36
# speedup vs baseline: 1.1427x; 1.1427x over previous
"""HAN layer (3-metapath GraphConv + semantic attention) on 8 trn2 NeuronCores.

Strategy (per sharding hint): shard destination nodes across the 8 cores
(6250 rows each), partition each metapath's edge list by destination shard
on the host, and sort/pad it into fixed-size 128-edge chunks per 128-dst
output tile.  Each core gathers source rows of h (stored bf16, replicated
in every core's DRAM) with the vectorized SWDGE ``dma_gather`` instruction
(<=1024 indices per call -- larger calls overflow the SWDGE descriptor
ring; src ids are split at 32768 into two base pointers because the
instruction's indices are int16), multiplies each gathered 128-edge chunk
with a host-precomputed selection matrix S (streamed from DRAM in bf16;
S[edge, dst_rel] = norm weight) on the tensor engine, accumulating z^T
tiles in PSUM.  A tiny [1,3] per-metapath score vector is all-reduced
across cores for the softmax semantic attention, then each core writes its
6250-row slice of the output.
"""

import numpy as np
import ml_dtypes

import concourse.bass as bass
import concourse.bacc as bacc
import concourse.mybir as mybir
import concourse.tile as tile
from concourse.bass_utils import run_bass_kernel_spmd

P = 128
N = 50000
D = 128
M = 3
E = 1_600_000
NCORES = 8
NSH = N // NCORES          # 6250 dst rows per core
NTILES = (NSH + P - 1) // P  # 49 output tiles (last has 106 real rows)
SPLIT = 32768              # int16 gather-index limit
GMAX = 8                   # max chunks (1024 idxs) per dma_gather call

TRACE = False
LAST_RESULTS = None

_PROGRAM_CACHE = {}


def _wrap_idx(idx):
    """[n] int16 -> [128, n//16] SWDGE index layout: position j at
    [j % 16, j // 16], replicated across the 8 groups of 16 partitions."""
    n = idx.shape[0]
    arr = idx.reshape(n // 16, 16).T  # [16, n//16]
    return np.tile(arr, (8, 1))


def _preprocess(edges):
    """Host-side: per-core gather-index streams and selection matrices.

    Per core, per dst tile t (128 dst rows), edges of all 3 metapaths are
    split into src<SPLIT ("lo") and src>=SPLIT ("hi") halves, each half
    grouped per metapath, sorted by src, and padded to a COMMON
    (max-over-cores) multiple-of-128 segment size so the SPMD program's
    layout is identical on every core (pad idx 0, wgt 0).
    Chunk order within a tile: [lo m0, lo m1, lo m2, hi m0, hi m1, hi m2].

    Returns (idx_all, s_all, cfg):
      idx_all[core]: [128, NTILES*W_IDX] int16, per-t window holds the
        wrapped lo indices then hi indices (hi stored as src-SPLIT).
      s_all[core]: [128, NTILES*NCH*128] bf16 selection matrices;
        chunk c of tile t at cols (t*NCH+c)*128: S[p, drel] = wgt for
        edge (c*128+p).
      cfg: (NCH, W_IDX, seg chunk counts per (t, half, m)).
    """
    per_core = [[None] * M for _ in range(NCORES)]
    for m in range(M):
        src = np.asarray(edges[m, 0])
        dst = np.asarray(edges[m, 1])
        out_deg = np.bincount(src, minlength=N).astype(np.float32)
        in_deg = np.bincount(dst, minlength=N).astype(np.float32)
        ns = 1.0 / np.sqrt(np.maximum(out_deg, 1.0))
        nd = 1.0 / np.sqrt(np.maximum(in_deg, 1.0))
        w_e = (ns[src] * nd[dst]).astype(np.float32)

        order = np.lexsort((src, dst))
        src_s, dst_s, w_s = src[order], dst[order], w_e[order]
        shard_bounds = np.searchsorted(dst_s, NSH * np.arange(NCORES + 1))
        for core in range(NCORES):
            lo, hi = shard_bounds[core], shard_bounds[core + 1]
            per_core[core][m] = (
                src_s[lo:hi],
                dst_s[lo:hi] - core * NSH,
                w_s[lo:hi],
            )

    # Raw per (core, t, half, m) segments: unique sorted src ids to gather,
    # plus per-edge (position-in-unique, drel, wgt) for the S scatter.
    segs = {}
    seg_n = np.zeros((NCORES, NTILES, 2, M), np.int64)
    for core in range(NCORES):
        for m in range(M):
            sc, dc, wc = per_core[core][m]
            tile_id = dc >> 7
            tb = np.searchsorted(tile_id, np.arange(NTILES + 1))
            for t in range(NTILES):
                s = slice(tb[t], tb[t + 1])
                st, dt_, wt = sc[s], dc[s] - t * P, wc[s]
                lo_mask = st < SPLIT
                for half in range(2):
                    mk = lo_mask if half == 0 else ~lo_mask
                    ss, ds, ws = st[mk], dt_[mk], wt[mk]
                    uniq, inv = np.unique(ss, return_inverse=True)
                    segs[(core, t, half, m)] = (uniq, inv, ds, ws)
                    seg_n[core, t, half, m] = uniq.shape[0]

    # Common (max over cores) chunk count per (t, half, m).
    segc = -(-seg_n.max(axis=0) // P)  # [NTILES, 2, M] ceil
    NCH = int(segc.sum(axis=(1, 2)).max())
    nidx_t = segc.sum(axis=(1, 2)) * P
    W_IDX = int(nidx_t.max()) // 16
    idx_all, s_all = [], []
    for core in range(NCORES):
        idx_arr = np.zeros((128, NTILES * W_IDX), np.int16)
        s_arr = np.zeros((128, NTILES * NCH * 128), ml_dtypes.bfloat16)
        for t in range(NTILES):
            parts = []
            c0 = 0
            for half in range(2):
                for m in range(M):
                    uniq, inv, ds, ws = segs[(core, t, half, m)]
                    ncol = int(segc[t, half, m])
                    npad = ncol * P - uniq.shape[0]
                    base = 0 if half == 0 else SPLIT
                    up = np.concatenate(
                        [uniq, np.full(npad, base, uniq.dtype)]
                    ).astype(np.int64)
                    parts.append((up - base).astype(np.int16))
                    # S scatter: unique-pos q -> chunk q//128, lane q%128;
                    # duplicate (src, dst) edges accumulate.
                    seg = np.zeros((128, ncol * 128), np.float32)
                    np.add.at(seg, (inv % P, (inv // P) * P + ds), ws)
                    s_arr[
                        :,
                        (t * NCH + c0) * 128 : (t * NCH + c0 + ncol) * 128,
                    ] = seg.astype(ml_dtypes.bfloat16)
                    c0 += ncol
            flat = np.concatenate(parts)
            idx_arr[:, t * W_IDX : t * W_IDX + flat.shape[0] // 16] = _wrap_idx(
                flat
            )
        idx_all.append(np.ascontiguousarray(idx_arr))
        s_all.append(np.ascontiguousarray(s_arr))

    cfg = (
        NCH,
        W_IDX,
        tuple(
            tuple(tuple(int(x) for x in segc[t, h]) for h in range(2))
            for t in range(NTILES)
        ),
    )
    return idx_all, s_all, cfg


def _build_program(cfg):
    if cfg in _PROGRAM_CACHE:
        return _PROGRAM_CACHE[cfg]
    f32 = mybir.dt.float32
    bf16 = mybir.dt.bfloat16
    NCH, W_IDX, segc_t = cfg
    nlo_t = [sum(segc_t[t][0]) * P for t in range(NTILES)]
    nhi_t = [sum(segc_t[t][1]) * P for t in range(NTILES)]

    nc = bacc.Bacc(
        None, target_bir_lowering=False, num_devices=NCORES, num_swdge_queues=4
    )
    h_d = nc.dram_tensor("h", [N, D], bf16, kind="ExternalInput")
    idx_d = nc.dram_tensor(
        "idx", [128, NTILES * W_IDX], mybir.dt.int16, kind="ExternalInput"
    )
    s_d = nc.dram_tensor(
        "s", [128, NTILES * NCH * 128], bf16, kind="ExternalInput"
    )
    ident_d = nc.dram_tensor("ident", [P, P], bf16, kind="ExternalInput")
    w1_d = nc.dram_tensor("w1", [D, D], bf16, kind="ExternalInput")
    w2_d = nc.dram_tensor("w2", [D, 1], bf16, kind="ExternalInput")
    b1_d = nc.dram_tensor("b1", [D, 1], f32, kind="ExternalInput")
    out_d = nc.dram_tensor("out", [NSH, D], f32, kind="ExternalOutput")

    with tile.TileContext(nc) as tc:
        with (
            tc.tile_pool(name="consts", bufs=1) as cpool,
            tc.tile_pool(name="zpool", bufs=1) as zpool,
            tc.tile_pool(name="score_psum", bufs=1, space="PSUM") as sp,
        ):
          with (
            tc.tile_pool(name="meta", bufs=3) as mpool,
            tc.tile_pool(name="gather", bufs=2) as gpool,
            tc.tile_pool(name="sel", bufs=2) as spool,
            tc.tile_pool(name="psum_z", bufs=2, space="PSUM") as pz,
            tc.tile_pool(name="att_psum", bufs=2, space="PSUM") as ap_,
            tc.tile_pool(name="att_sbuf", bufs=2) as asb,
          ):
            ident_sb = cpool.tile([P, P], bf16)
            nc.sync.dma_start(out=ident_sb[:], in_=ident_d[:])
            w1_sb = cpool.tile([P, P], bf16)
            nc.sync.dma_start(out=w1_sb[:], in_=w1_d[:])
            w2_sb = cpool.tile([P, 1], bf16)
            nc.sync.dma_start(out=w2_sb[:], in_=w2_d[:])
            b1_sb = cpool.tile([P, 1], f32)
            nc.sync.dma_start(out=b1_sb[:], in_=b1_d[:])

            # z stored d-major: z^T[m] tile t lives at columns (m*NTILES+t)*P
            z_sb = zpool.tile([P, M * NTILES * P], bf16)

            psum_s = sp.tile([1, M * P], f32, space="PSUM")
            qrr = [0]  # SWDGE queue round-robin counter
            # ---- sparse aggregation ----
            for t in range(NTILES):
                nlo, nhi = nlo_t[t], nhi_t[t]
                nch = (nlo + nhi) // P
                i_sb = mpool.tile([128, W_IDX], mybir.dt.int16, tag="idx")
                nc.sync.dma_start(
                    out=i_sb[:], in_=idx_d[:, t * W_IDX : (t + 1) * W_IDX]
                )
                s_sb = spool.tile([128, NCH * 128], bf16, tag="s")
                nc.sync.dma_start(
                    out=s_sb[:, : nch * 128],
                    in_=s_d[:, t * NCH * 128 : t * NCH * 128 + nch * 128],
                )
                g_sb = gpool.tile([128, nch, P], bf16, tag="g")
                # dma_gather is limited to 1024 indices (8 chunks) per call:
                # larger calls overflow the SWDGE descriptor ring and wedge
                # the device.  Round-robin the 4 SWDGE queues.
                for half, base_ap, ch0, nch_h in (
                    (0, h_d[:, :], 0, nlo // P),
                    (1, h_d[SPLIT:, :], nlo // P, nhi // P),
                ):
                    for off in range(0, nch_h, GMAX):
                        k = min(GMAX, nch_h - off)
                        c = ch0 + off
                        nc.gpsimd.dma_gather(
                            g_sb[:, c : c + k, :],
                            base_ap,
                            i_sb[:, c * 8 : c * 8 + k * 8],
                            k * P,
                            k * P,
                            P,
                            queue_num=qrr[0] % 4,
                        )
                        qrr[0] += 1
                # chunk columns for (half, m) segments in order
                seg_off = {}
                c0 = 0
                for half in range(2):
                    for m in range(M):
                        seg_off[(half, m)] = c0
                        c0 += segc_t[t][half][m]
                for m in range(M):
                    cols = list(
                        range(
                            seg_off[(0, m)],
                            seg_off[(0, m)] + segc_t[t][0][m],
                        )
                    ) + list(
                        range(
                            seg_off[(1, m)],
                            seg_off[(1, m)] + segc_t[t][1][m],
                        )
                    )
                    psum_zt = pz.tile([P, P], f32, space="PSUM", tag="zt")
                    for k, c in enumerate(cols):
                        nc.tensor.matmul(
                            out=psum_zt[:],
                            lhsT=g_sb[:, c, :],
                            rhs=s_sb[:, c * 128 : (c + 1) * 128],
                            start=(k == 0),
                            stop=(k == len(cols) - 1),
                        )
                    zt = z_sb[
                        :, (m * NTILES + t) * P : (m * NTILES + t + 1) * P
                    ]
                    nc.scalar.copy(out=zt, in_=psum_zt[:])
                    # fold the semantic-attention score accumulation in so
                    # it overlaps the gather-bound main loop
                    psum_y = ap_.tile([P, P], f32, space="PSUM", tag="y")
                    nc.tensor.matmul(
                        out=psum_y[:], lhsT=w1_sb[:], rhs=zt, start=True,
                        stop=True,
                    )
                    tanh_sb = asb.tile([P, P], bf16, tag="tanh")
                    nc.scalar.activation(
                        out=tanh_sb[:],
                        in_=psum_y[:],
                        func=mybir.ActivationFunctionType.Tanh,
                        bias=b1_sb[:, :1],
                    )
                    nc.tensor.matmul(
                        out=psum_s[:, m * P : (m + 1) * P],
                        lhsT=w2_sb[:],
                        rhs=tanh_sb[:],
                        start=(t == 0),
                        stop=(t == NTILES - 1),
                        skip_group_check=True,
                    )

          # ---- semantic attention epilogue ----
          with (
            tc.tile_pool(name="out_psum", bufs=2, space="PSUM") as op_,
            tc.tile_pool(name="epi_sbuf", bufs=2) as es,
            tc.tile_pool(name="small", bufs=1) as sm,
          ):
            wrow = sm.tile([1, M], f32)
            for m in range(M):
                nc.vector.reduce_sum(
                    out=wrow[:, m : m + 1],
                    in_=psum_s[:, m * P : (m + 1) * P],
                    axis=mybir.AxisListType.X,
                )
            with tc.tile_pool(name="ccdram", bufs=1, space="DRAM") as ccp:
                cc_in_t = ccp.tile([1, M], f32)
                cc_out_t = ccp.tile([1, M], f32, addr_space="Shared")
                nc.gpsimd.dma_start(cc_in_t[:], wrow[:])
                nc.gpsimd.collective_compute(
                    "AllReduce",
                    mybir.AluOpType.add,
                    replica_groups=[list(range(NCORES))],
                    ins=[cc_in_t.opt()],
                    outs=[cc_out_t.opt()],
                )
                w_bc = sm.tile([P, M], f32)
                nc.sync.dma_start(
                    out=w_bc[:], in_=cc_out_t[0:1, :].to_broadcast([P, M])
                )
            # softmax over the M columns (identical on every partition)
            nc.vector.tensor_scalar(
                out=w_bc[:],
                in0=w_bc[:],
                scalar1=1.0 / N,
                scalar2=None,
                op0=mybir.AluOpType.mult,
            )
            negmax = sm.tile([P, 1], f32)
            nc.vector.tensor_reduce(
                out=negmax[:],
                in_=w_bc[:],
                axis=mybir.AxisListType.X,
                op=mybir.AluOpType.max,
                negate=True,
            )
            e_bc = sm.tile([P, M], f32)
            nc.scalar.activation(
                out=e_bc[:],
                in_=w_bc[:],
                func=mybir.ActivationFunctionType.Exp,
                bias=negmax[:, :1],
            )
            esum = sm.tile([P, 1], f32)
            nc.vector.reduce_sum(
                out=esum[:], in_=e_bc[:], axis=mybir.AxisListType.X
            )
            rsum = sm.tile([P, 1], f32)
            nc.vector.reciprocal(out=rsum[:], in_=esum[:])
            beta = sm.tile([P, M], f32)
            nc.vector.tensor_scalar(
                out=beta[:],
                in0=e_bc[:],
                scalar1=rsum[:, :1],
                scalar2=None,
                op0=mybir.AluOpType.mult,
            )
            ibeta = sm.tile([P, M * P], bf16)
            for m in range(M):
                nc.vector.tensor_scalar(
                    out=ibeta[:, m * P : (m + 1) * P],
                    in0=ident_sb[:],
                    scalar1=beta[:, m : m + 1],
                    scalar2=None,
                    op0=mybir.AluOpType.mult,
                )
            # ---- final combine: out tile = sum_m z_m^T(tile)^T @ (I * beta_m) ----
            for t in range(NTILES):
                psum_o = op_.tile([P, P], f32, space="PSUM", tag="o")
                for m in range(M):
                    nc.tensor.matmul(
                        out=psum_o[:],
                        lhsT=z_sb[:, (m * NTILES + t) * P : (m * NTILES + t + 1) * P],
                        rhs=ibeta[:, m * P : (m + 1) * P],
                        start=(m == 0),
                        stop=(m == M - 1),
                    )
                rows = min(P, NSH - t * P)
                o_sb = es.tile([P, P], f32, tag="out")
                nc.vector.tensor_copy(out=o_sb[:], in_=psum_o[:])
                nc.sync.dma_start(
                    out=out_d[t * P : t * P + rows, :], in_=o_sb[:rows, :]
                )
    nc.finalize()
    _PROGRAM_CACHE[cfg] = nc
    return nc


def kernel(h, edges, W1, b1, W2):
    global LAST_RESULTS
    h = np.asarray(h, dtype=np.float32)
    h_bf = np.ascontiguousarray(h.astype(ml_dtypes.bfloat16))
    edges = np.asarray(edges)
    idx_all, s_all, cfg = _preprocess(edges)
    nc = _build_program(cfg)

    ident = np.eye(P, dtype=np.float32).astype(ml_dtypes.bfloat16)
    w1 = np.ascontiguousarray(
        np.asarray(W1, dtype=np.float32).astype(ml_dtypes.bfloat16)
    )
    w2 = np.ascontiguousarray(
        np.asarray(W2, dtype=np.float32).reshape(D, 1).astype(ml_dtypes.bfloat16)
    )
    b1c = np.ascontiguousarray(np.asarray(b1, dtype=np.float32).reshape(D, 1))

    in_maps = []
    for core in range(NCORES):
        in_maps.append(
            {
                "h": h_bf,
                "idx": idx_all[core],
                "s": s_all[core],
                "ident": ident,
                "w1": w1,
                "w2": w2,
                "b1": b1c,
            }
        )
    res = run_bass_kernel_spmd(
        nc, in_maps, core_ids=list(range(NCORES)), trace=TRACE
    )
    LAST_RESULTS = res
    out = np.concatenate([res.results[c]["out"] for c in range(NCORES)], axis=0)
    return out


# revision 37
# speedup vs baseline: 1.1517x; 1.0079x over previous
"""HAN layer (3-metapath GraphConv + semantic attention) on 8 trn2 NeuronCores.

Strategy (per sharding hint): shard destination nodes across the 8 cores
(6250 rows each), partition each metapath's edge list by destination shard
on the host, and sort/pad it into fixed-size 128-edge chunks per 128-dst
output tile.  Each core gathers source rows of h (stored bf16, replicated
in every core's DRAM) with the vectorized SWDGE ``dma_gather`` instruction
(<=1024 indices per call -- larger calls overflow the SWDGE descriptor
ring; src ids are split at 32768 into two base pointers because the
instruction's indices are int16), multiplies each gathered 128-edge chunk
with a host-precomputed selection matrix S (streamed from DRAM in bf16;
S[edge, dst_rel] = norm weight) on the tensor engine, accumulating z^T
tiles in PSUM.  A tiny [1,3] per-metapath score vector is all-reduced
across cores for the softmax semantic attention, then each core writes its
6250-row slice of the output.
"""

import numpy as np
import ml_dtypes

import concourse.bass as bass
import concourse.bacc as bacc
import concourse.mybir as mybir
import concourse.tile as tile
from concourse.bass_utils import run_bass_kernel_spmd

P = 128
N = 50000
D = 128
M = 3
E = 1_600_000
NCORES = 8
NSH = N // NCORES          # 6250 dst rows per core
NTILES = (NSH + P - 1) // P  # 49 output tiles (last has 106 real rows)
SPLIT = 32768              # int16 gather-index limit
GMAX = 8                   # max chunks (1024 idxs) per dma_gather call

TRACE = False
LAST_RESULTS = None

_PROGRAM_CACHE = {}


def _wrap_idx(idx):
    """[n] int16 -> [128, n//16] SWDGE index layout: position j at
    [j % 16, j // 16], replicated across the 8 groups of 16 partitions."""
    n = idx.shape[0]
    arr = idx.reshape(n // 16, 16).T  # [16, n//16]
    return np.tile(arr, (8, 1))


def _preprocess(edges):
    """Host-side: per-core gather-index streams and selection matrices.

    Per core, per dst tile t (128 dst rows), edges of all 3 metapaths are
    split into src<SPLIT ("lo") and src>=SPLIT ("hi") halves, each half
    grouped per metapath, sorted by src, and padded to a COMMON
    (max-over-cores) multiple-of-128 segment size so the SPMD program's
    layout is identical on every core (pad idx 0, wgt 0).
    Chunk order within a tile: [lo m0, lo m1, lo m2, hi m0, hi m1, hi m2].

    Returns (idx_all, s_all, cfg):
      idx_all[core]: [128, NTILES*W_IDX] int16, per-t window holds the
        wrapped lo indices then hi indices (hi stored as src-SPLIT).
      s_all[core]: [128, NTILES*NCH*128] bf16 selection matrices;
        chunk c of tile t at cols (t*NCH+c)*128: S[p, drel] = wgt for
        edge (c*128+p).
      cfg: (NCH, W_IDX, seg chunk counts per (t, half, m)).
    """
    per_core = [[None] * M for _ in range(NCORES)]
    for m in range(M):
        src = np.asarray(edges[m, 0])
        dst = np.asarray(edges[m, 1])
        out_deg = np.bincount(src, minlength=N).astype(np.float32)
        in_deg = np.bincount(dst, minlength=N).astype(np.float32)
        ns = 1.0 / np.sqrt(np.maximum(out_deg, 1.0))
        nd = 1.0 / np.sqrt(np.maximum(in_deg, 1.0))
        w_e = (ns[src] * nd[dst]).astype(np.float32)

        order = np.lexsort((src, dst))
        src_s, dst_s, w_s = src[order], dst[order], w_e[order]
        shard_bounds = np.searchsorted(dst_s, NSH * np.arange(NCORES + 1))
        for core in range(NCORES):
            lo, hi = shard_bounds[core], shard_bounds[core + 1]
            per_core[core][m] = (
                src_s[lo:hi],
                dst_s[lo:hi] - core * NSH,
                w_s[lo:hi],
            )

    # Raw per (core, t, half, m) segments: unique sorted src ids to gather,
    # plus per-edge (position-in-unique, drel, wgt) for the S scatter.
    segs = {}
    seg_n = np.zeros((NCORES, NTILES, 2, M), np.int64)
    for core in range(NCORES):
        for m in range(M):
            sc, dc, wc = per_core[core][m]
            tile_id = dc >> 7
            tb = np.searchsorted(tile_id, np.arange(NTILES + 1))
            for t in range(NTILES):
                s = slice(tb[t], tb[t + 1])
                st, dt_, wt = sc[s], dc[s] - t * P, wc[s]
                lo_mask = st < SPLIT
                for half in range(2):
                    mk = lo_mask if half == 0 else ~lo_mask
                    ss, ds, ws = st[mk], dt_[mk], wt[mk]
                    uniq, inv = np.unique(ss, return_inverse=True)
                    segs[(core, t, half, m)] = (uniq, inv, ds, ws)
                    seg_n[core, t, half, m] = uniq.shape[0]

    # Common (max over cores) chunk count per (t, half, m).
    segc = -(-seg_n.max(axis=0) // P)  # [NTILES, 2, M] ceil
    NCH = int(segc.sum(axis=(1, 2)).max())
    nidx_t = segc.sum(axis=(1, 2)) * P
    W_IDX = int(nidx_t.max()) // 16
    idx_all, s_all = [], []
    for core in range(NCORES):
        idx_arr = np.zeros((128, NTILES * W_IDX), np.int16)
        s_arr = np.zeros((128, NTILES * NCH * 128), ml_dtypes.bfloat16)
        for t in range(NTILES):
            parts = []
            c0 = 0
            for half in range(2):
                for m in range(M):
                    uniq, inv, ds, ws = segs[(core, t, half, m)]
                    ncol = int(segc[t, half, m])
                    npad = ncol * P - uniq.shape[0]
                    base = 0 if half == 0 else SPLIT
                    up = np.concatenate(
                        [uniq, np.full(npad, base, uniq.dtype)]
                    ).astype(np.int64)
                    parts.append((up - base).astype(np.int16))
                    # S scatter: unique-pos q -> chunk q//128, lane q%128;
                    # duplicate (src, dst) edges accumulate.
                    seg = np.zeros((128, ncol * 128), np.float32)
                    np.add.at(seg, (inv % P, (inv // P) * P + ds), ws)
                    s_arr[
                        :,
                        (t * NCH + c0) * 128 : (t * NCH + c0 + ncol) * 128,
                    ] = seg.astype(ml_dtypes.bfloat16)
                    c0 += ncol
            flat = np.concatenate(parts)
            idx_arr[:, t * W_IDX : t * W_IDX + flat.shape[0] // 16] = _wrap_idx(
                flat
            )
        idx_all.append(np.ascontiguousarray(idx_arr))
        s_all.append(np.ascontiguousarray(s_arr))

    cfg = (
        NCH,
        W_IDX,
        tuple(
            tuple(tuple(int(x) for x in segc[t, h]) for h in range(2))
            for t in range(NTILES)
        ),
    )
    return idx_all, s_all, cfg


def _build_program(cfg):
    if cfg in _PROGRAM_CACHE:
        return _PROGRAM_CACHE[cfg]
    f32 = mybir.dt.float32
    bf16 = mybir.dt.bfloat16
    NCH, W_IDX, segc_t = cfg
    nlo_t = [sum(segc_t[t][0]) * P for t in range(NTILES)]
    nhi_t = [sum(segc_t[t][1]) * P for t in range(NTILES)]

    nc = bacc.Bacc(
        None, target_bir_lowering=False, num_devices=NCORES, num_swdge_queues=4
    )
    h_d = nc.dram_tensor("h", [N, D], bf16, kind="ExternalInput")
    idx_d = nc.dram_tensor(
        "idx", [128, NTILES * W_IDX], mybir.dt.int16, kind="ExternalInput"
    )
    s_d = nc.dram_tensor(
        "s", [128, NTILES * NCH * 128], bf16, kind="ExternalInput"
    )
    ident_d = nc.dram_tensor("ident", [P, P], bf16, kind="ExternalInput")
    w1_d = nc.dram_tensor("w1", [D, D], bf16, kind="ExternalInput")
    w2_d = nc.dram_tensor("w2", [D, 1], bf16, kind="ExternalInput")
    b1_d = nc.dram_tensor("b1", [D, 1], f32, kind="ExternalInput")
    out_d = nc.dram_tensor("out", [NSH, D], f32, kind="ExternalOutput")

    with tile.TileContext(nc) as tc:
        with (
            tc.tile_pool(name="consts", bufs=1) as cpool,
            tc.tile_pool(name="zpool", bufs=1) as zpool,
            tc.tile_pool(name="score_psum", bufs=1, space="PSUM") as sp,
        ):
          with (
            tc.tile_pool(name="meta", bufs=3) as mpool,
            tc.tile_pool(name="gather", bufs=3) as gpool,
            tc.tile_pool(name="sel", bufs=2) as spool,
            tc.tile_pool(name="psum_z", bufs=2, space="PSUM") as pz,
            tc.tile_pool(name="att_psum", bufs=2, space="PSUM") as ap_,
            tc.tile_pool(name="att_sbuf", bufs=2) as asb,
          ):
            ident_sb = cpool.tile([P, P], bf16)
            nc.sync.dma_start(out=ident_sb[:], in_=ident_d[:])
            w1_sb = cpool.tile([P, P], bf16)
            nc.sync.dma_start(out=w1_sb[:], in_=w1_d[:])
            w2_sb = cpool.tile([P, 1], bf16)
            nc.sync.dma_start(out=w2_sb[:], in_=w2_d[:])
            b1_sb = cpool.tile([P, 1], f32)
            nc.sync.dma_start(out=b1_sb[:], in_=b1_d[:])

            # z stored d-major: z^T[m] tile t lives at columns (m*NTILES+t)*P
            z_sb = zpool.tile([P, M * NTILES * P], bf16)

            psum_s = sp.tile([1, M * P], f32, space="PSUM")
            qrr = [0]  # SWDGE queue round-robin counter
            # ---- sparse aggregation ----
            for t in range(NTILES):
                nlo, nhi = nlo_t[t], nhi_t[t]
                nch = (nlo + nhi) // P
                i_sb = mpool.tile([128, W_IDX], mybir.dt.int16, tag="idx")
                nc.sync.dma_start(
                    out=i_sb[:], in_=idx_d[:, t * W_IDX : (t + 1) * W_IDX]
                )
                s_sb = spool.tile([128, NCH * 128], bf16, tag="s")
                nc.sync.dma_start(
                    out=s_sb[:, : nch * 128],
                    in_=s_d[:, t * NCH * 128 : t * NCH * 128 + nch * 128],
                )
                g_sb = gpool.tile([128, nch, P], bf16, tag="g")
                # dma_gather is limited to 1024 indices (8 chunks) per call:
                # larger calls overflow the SWDGE descriptor ring and wedge
                # the device.  Round-robin the 4 SWDGE queues.
                for half, base_ap, ch0, nch_h in (
                    (0, h_d[:, :], 0, nlo // P),
                    (1, h_d[SPLIT:, :], nlo // P, nhi // P),
                ):
                    for off in range(0, nch_h, GMAX):
                        k = min(GMAX, nch_h - off)
                        c = ch0 + off
                        nc.gpsimd.dma_gather(
                            g_sb[:, c : c + k, :],
                            base_ap,
                            i_sb[:, c * 8 : c * 8 + k * 8],
                            k * P,
                            k * P,
                            P,
                            queue_num=qrr[0] % 4,
                        )
                        qrr[0] += 1
                # chunk columns for (half, m) segments in order
                seg_off = {}
                c0 = 0
                for half in range(2):
                    for m in range(M):
                        seg_off[(half, m)] = c0
                        c0 += segc_t[t][half][m]
                for m in range(M):
                    cols = list(
                        range(
                            seg_off[(0, m)],
                            seg_off[(0, m)] + segc_t[t][0][m],
                        )
                    ) + list(
                        range(
                            seg_off[(1, m)],
                            seg_off[(1, m)] + segc_t[t][1][m],
                        )
                    )
                    psum_zt = pz.tile([P, P], f32, space="PSUM", tag="zt")
                    for k, c in enumerate(cols):
                        nc.tensor.matmul(
                            out=psum_zt[:],
                            lhsT=g_sb[:, c, :],
                            rhs=s_sb[:, c * 128 : (c + 1) * 128],
                            start=(k == 0),
                            stop=(k == len(cols) - 1),
                        )
                    zt = z_sb[
                        :, (m * NTILES + t) * P : (m * NTILES + t + 1) * P
                    ]
                    nc.scalar.copy(out=zt, in_=psum_zt[:])
                    # fold the semantic-attention score accumulation in so
                    # it overlaps the gather-bound main loop
                    psum_y = ap_.tile([P, P], f32, space="PSUM", tag="y")
                    nc.tensor.matmul(
                        out=psum_y[:], lhsT=w1_sb[:], rhs=zt, start=True,
                        stop=True,
                    )
                    tanh_sb = asb.tile([P, P], bf16, tag="tanh")
                    nc.scalar.activation(
                        out=tanh_sb[:],
                        in_=psum_y[:],
                        func=mybir.ActivationFunctionType.Tanh,
                        bias=b1_sb[:, :1],
                    )
                    nc.tensor.matmul(
                        out=psum_s[:, m * P : (m + 1) * P],
                        lhsT=w2_sb[:],
                        rhs=tanh_sb[:],
                        start=(t == 0),
                        stop=(t == NTILES - 1),
                        skip_group_check=True,
                    )

          # ---- semantic attention epilogue ----
          with (
            tc.tile_pool(name="out_psum", bufs=2, space="PSUM") as op_,
            tc.tile_pool(name="epi_sbuf", bufs=2) as es,
            tc.tile_pool(name="small", bufs=1) as sm,
          ):
            wrow = sm.tile([1, M], f32)
            for m in range(M):
                nc.vector.reduce_sum(
                    out=wrow[:, m : m + 1],
                    in_=psum_s[:, m * P : (m + 1) * P],
                    axis=mybir.AxisListType.X,
                )
            with tc.tile_pool(name="ccdram", bufs=1, space="DRAM") as ccp:
                cc_in_t = ccp.tile([1, M], f32)
                cc_out_t = ccp.tile([1, M], f32, addr_space="Shared")
                nc.gpsimd.dma_start(cc_in_t[:], wrow[:])
                nc.gpsimd.collective_compute(
                    "AllReduce",
                    mybir.AluOpType.add,
                    replica_groups=[list(range(NCORES))],
                    ins=[cc_in_t.opt()],
                    outs=[cc_out_t.opt()],
                )
                w_bc = sm.tile([P, M], f32)
                nc.sync.dma_start(
                    out=w_bc[:], in_=cc_out_t[0:1, :].to_broadcast([P, M])
                )
            # softmax over the M columns (identical on every partition)
            nc.vector.tensor_scalar(
                out=w_bc[:],
                in0=w_bc[:],
                scalar1=1.0 / N,
                scalar2=None,
                op0=mybir.AluOpType.mult,
            )
            negmax = sm.tile([P, 1], f32)
            nc.vector.tensor_reduce(
                out=negmax[:],
                in_=w_bc[:],
                axis=mybir.AxisListType.X,
                op=mybir.AluOpType.max,
                negate=True,
            )
            e_bc = sm.tile([P, M], f32)
            nc.scalar.activation(
                out=e_bc[:],
                in_=w_bc[:],
                func=mybir.ActivationFunctionType.Exp,
                bias=negmax[:, :1],
            )
            esum = sm.tile([P, 1], f32)
            nc.vector.reduce_sum(
                out=esum[:], in_=e_bc[:], axis=mybir.AxisListType.X
            )
            rsum = sm.tile([P, 1], f32)
            nc.vector.reciprocal(out=rsum[:], in_=esum[:])
            beta = sm.tile([P, M], f32)
            nc.vector.tensor_scalar(
                out=beta[:],
                in0=e_bc[:],
                scalar1=rsum[:, :1],
                scalar2=None,
                op0=mybir.AluOpType.mult,
            )
            ibeta = sm.tile([P, M * P], bf16)
            for m in range(M):
                nc.vector.tensor_scalar(
                    out=ibeta[:, m * P : (m + 1) * P],
                    in0=ident_sb[:],
                    scalar1=beta[:, m : m + 1],
                    scalar2=None,
                    op0=mybir.AluOpType.mult,
                )
            # ---- final combine: out tile = sum_m z_m^T(tile)^T @ (I * beta_m) ----
            for t in range(NTILES):
                psum_o = op_.tile([P, P], f32, space="PSUM", tag="o")
                for m in range(M):
                    nc.tensor.matmul(
                        out=psum_o[:],
                        lhsT=z_sb[:, (m * NTILES + t) * P : (m * NTILES + t + 1) * P],
                        rhs=ibeta[:, m * P : (m + 1) * P],
                        start=(m == 0),
                        stop=(m == M - 1),
                    )
                rows = min(P, NSH - t * P)
                o_sb = es.tile([P, P], f32, tag="out")
                nc.vector.tensor_copy(out=o_sb[:], in_=psum_o[:])
                nc.sync.dma_start(
                    out=out_d[t * P : t * P + rows, :], in_=o_sb[:rows, :]
                )
    nc.finalize()
    _PROGRAM_CACHE[cfg] = nc
    return nc


def kernel(h, edges, W1, b1, W2):
    global LAST_RESULTS
    h = np.asarray(h, dtype=np.float32)
    h_bf = np.ascontiguousarray(h.astype(ml_dtypes.bfloat16))
    edges = np.asarray(edges)
    idx_all, s_all, cfg = _preprocess(edges)
    nc = _build_program(cfg)

    ident = np.eye(P, dtype=np.float32).astype(ml_dtypes.bfloat16)
    w1 = np.ascontiguousarray(
        np.asarray(W1, dtype=np.float32).astype(ml_dtypes.bfloat16)
    )
    w2 = np.ascontiguousarray(
        np.asarray(W2, dtype=np.float32).reshape(D, 1).astype(ml_dtypes.bfloat16)
    )
    b1c = np.ascontiguousarray(np.asarray(b1, dtype=np.float32).reshape(D, 1))

    in_maps = []
    for core in range(NCORES):
        in_maps.append(
            {
                "h": h_bf,
                "idx": idx_all[core],
                "s": s_all[core],
                "ident": ident,
                "w1": w1,
                "w2": w2,
                "b1": b1c,
            }
        )
    res = run_bass_kernel_spmd(
        nc, in_maps, core_ids=list(range(NCORES)), trace=TRACE
    )
    LAST_RESULTS = res
    out = np.concatenate([res.results[c]["out"] for c in range(NCORES)], axis=0)
    return out
